# revision 25
# baseline (speedup 1.0000x reference)
"""Trainium2 Bass kernel for nn_GatedAttention (linear attention with sigmoid
gate).

Strategy: shard the 16384 token rows across 8 cores (2048 each; cores 2b,2b+1
hold batch b). Per core, two phases:
  A: K,V projections (token-major) + per-head kv' = K^T [V|1] accumulated in
     persistent PSUM tiles over all local tokens (the ones column folds k_sum
     into kv'). kv matmuls run one m-tile behind the projections so the elu
     chain never stalls the PE.
  -- pairwise AllReduce of kv' between the two cores sharing a batch --
  B: Q,G projections (feature-major), out^T = kv'^T @ Q per head, normalizer
     z = SCALE/max(q.k_sum,eps) applied via tiny selector matmuls, gate, and
     the final output projection, all feature-major.
DMAs are issued in consumer order (X/wk/wv before phase A, wq/wg/wo/bg/sel
after the first m-tile, collective output after ch0's Q matmuls) so counting
semaphores never serialize the PE behind unrelated transfers.
Host transposes x to feature-major and pre-transposes weights; output returns
feature-major bf16 per-core slabs that the host transposes back.
"""
import sys

sys.path.insert(0, "/opt/trn_rl_repo")

import numpy as np
import ml_dtypes

B, N, DIM = 4, 4096, 1024
HEADS, DH = 16, 64
SCALE = DH ** -0.5
N_CORES = 8
TPC = B * N // N_CORES      # 2048 tokens per core
NMT = TPC // 128            # 16 m-tiles (phase A)
CHUNK = 512
NCH = TPC // CHUNK          # 4 chunks (phase B)
CLAMP = 1e-6  # ksd carries plain k_sum; SCALE applied in the zqr multiply

DT_MODE = "bf16"            # "bf16" | "f32r" | "f32"

_CACHE = {}


def _build(dt_mode=DT_MODE, reps=1):
    import concourse.bacc as bacc
    import concourse.bass as bass
    import concourse.tile as tile
    from concourse import mybir

    AF = mybir.ActivationFunctionType
    F32 = mybir.dt.float32
    F8 = mybir.dt.float8e4
    DR = mybir.MatmulPerfMode.DoubleRow
    DT = mybir.dt.bfloat16 if dt_mode == "bf16" else mybir.dt.float32

    def mm(ap):
        return ap.bitcast(mybir.dt.float32r) if dt_mode == "f32r" else ap

    ts = bass.ts

    nc = bacc.Bacc("TRN2", target_bir_lowering=False, debug=False,
                   num_devices=N_CORES)
    xt = nc.dram_tensor("xt", [DIM, TPC], DT, kind="ExternalInput")
    x8_d = nc.dram_tensor("x8", [DIM, TPC], F8, kind="ExternalInput")
    w_in = {}
    for nm in ("wv", "wo"):
        w_in[nm] = nc.dram_tensor(nm, [DIM, DIM], DT, kind="ExternalInput")
    for nm in ("wk8", "wq8", "wg8"):
        w_in[nm] = nc.dram_tensor(nm, [DIM, DIM], F8, kind="ExternalInput")
    bg_d = nc.dram_tensor("bg", [DIM], F32, kind="ExternalInput")
    y_d = nc.dram_tensor("y", [DIM, TPC], DT, kind="ExternalOutput")
    cc_in = nc.dram_tensor("cc_in", [128, 8, 65], F32)
    cc_out = nc.dram_tensor("cc_out", [128, 8, 65], F32)

    with tile.TileContext(nc, num_cores=N_CORES) as tc:
        with (
            tc.tile_pool(name="persist", bufs=1) as persist,
            tc.tile_pool(name="pb_big", bufs=2) as pb_big,
        ):
            X = persist.tile([128, 8, TPC], DT, tag="x")
            X8 = persist.tile([128, 8, TPC], F8, tag="x8")
            wsb = {}
            wsb["wo"] = persist.tile([128, 8, DIM], DT, tag="wo", name="wo")
            for nm in ("wq8", "wg8"):
                wsb[nm] = persist.tile([128, 8, DIM], F8, tag=nm, name=nm)
            bg_sb = persist.tile([128, 8], F32, tag="bg")
            sel_np = np.zeros((16, 8, 128), _np_dt(dt_mode))
            for p in range(8):
                sel_np[2 * p, p, 0:64] = 1.0
                sel_np[2 * p + 1, p, 64:128] = 1.0
            sel_d = nc.inline_tensor(sel_np, name="sel_const")
            sel = persist.tile([16, 8, 128], DT, tag="sel")

            for _rep in range(reps):
                _phases(nc, tc, bass, mybir, AF, F32, DT, mm, ts, X, wsb,
                        bg_sb, sel, sel_d, w_in, xt, bg_d, cc_in, cc_out, y_d,
                        tc_pools=(persist, pb_big), X8=X8, x8_d=x8_d,
                        F8=F8, DR=DR)
    nc.compile()
    return nc


def _phases(nc, tc, bass, mybir, AF, F32, DT, mm, ts, X, wsb, bg_sb, sel,
            sel_d, w_in, xt, bg_d, cc_in, cc_out, y_d, tc_pools, X8, x8_d,
            F8, DR):
    persist, pb_big = tc_pools
    # ---------------- phase A ----------------
    with (
        tc.tile_pool(name="pa_w", bufs=1) as pa_w,
        tc.tile_pool(name="pa_tmp", bufs=2) as pa_tmp,
        tc.tile_pool(name="pa_ps", bufs=3, space="PSUM") as pa_ps,
        tc.tile_pool(name="kv_ps", bufs=1, space="PSUM") as kv_pool,
    ):
        wsb["wk8"] = pa_w.tile([128, 8, DIM], F8, tag="wk8", name="wk8")
        wsb["wv"] = pa_w.tile([128, 8, DIM], DT, tag="wv", name="wv")
        # consumer-ordered DMA issue: phase A inputs first. Each input goes
        # as ONE large DMA (256KB chunks only reach ~64% of HBM bandwidth;
        # MB-size transfers reach ~85%+). The single HWDGE ring drains in
        # issue order at ~285GB/s with ~10.5us to first byte, so the order
        # IS the schedule: X8a+wk8 (2MB) unblock the K-projection run-ahead
        # at ~18us; Xh1+wv unblock V right as the K burst ends (~31us).
        def big_dma(out_ap, dram, ncols, c0, c1):
            src = bass.AP(tensor=dram.ap().tensor, offset=c0,
                          ap=[[ncols, 128], [128 * ncols, 8], [1, c1 - c0]])
            nc.sync.dma_start(out=out_ap, in_=src)

        big_dma(X8[:, :, 0:TPC // 2], x8_d, TPC, 0, TPC // 2)
        big_dma(wsb["wk8"][:, :, :], w_in["wk8"], DIM, 0, DIM)
        big_dma(X[:, :, 0:TPC // 2], xt, TPC, 0, TPC // 2)
        big_dma(wsb["wv"][:, :, :], w_in["wv"], DIM, 0, DIM)
        big_dma(X8[:, :, TPC // 2:TPC], x8_d, TPC, TPC // 2, TPC)
        big_dma(X[:, :, TPC // 2:TPC], xt, TPC, TPC // 2, TPC)

        # HAM warm-up + DMA-latency filler. Tiles come from pools that are
        # NEVER released mid-phase (persist / the pa_ps rotation): a
        # released warm pool's SBUF gets reused by pa_w, and the Tile
        # scheduler then gates the wk8 input DMA on the LAST dummy matmul
        # (WAR on the aliased range), stalling the whole input stream.
        wa = persist.tile([128, 128], DT, tag="wa")
        wb = persist.tile([128, 512], DT, tag="wb")
        nc.vector.memset(wa[:], 0.0)
        nc.vector.memset(wb[:], 0.0)
        wp = pa_ps.tile([128, 1024], F32, tag="proj")
        for _ in range(26):
            nc.tensor.matmul(wp[:, 0:512], mm(wa), mm(wb), start=True,
                             stop=True)

        # persistent PSUM accumulators for kv': 2 tiles x [128, 4, 128]
        # (tile w holds heads 8w..8w+7: slice j rows 0:64 = head 8w+2j,
        #  rows 64:128 = head 8w+2j+1; j-slices padded to 128 floats so a
        #  matmul output never crosses a PSUM bank boundary)
        kv_ps = [kv_pool.tile([128, 4, 128], F32, tag=f"kv{w}",
                              name=f"kv{w}") for w in range(2)]
        ksb_hist = [None] * NMT
        vp_hist = [None] * NMT

        def kv_mms(mt):
            ksb_o = ksb_hist[mt]
            vp_o = vp_hist[mt]
            for w in range(2):
                for j in range(4):
                    for c in range(2):
                        h = 8 * w + 2 * j + c
                        # start only on the FIRST matmul touching this bank's
                        # partition plane: start_tensor_calc marks the whole
                        # 2KB zero-region pending, so a second start=True in
                        # the same bank would re-poison already-written
                        # slices and turn later accumulates into overwrites.
                        nc.tensor.matmul(
                            kv_ps[w][64 * c:64 * c + 64, j, 0:65],
                            mm(ksb_o[:, ts(h, 64)]),
                            mm(vp_o[:, h, :]),
                            start=(mt == 0 and j == 0),
                            stop=(mt == NMT - 1),
                            skip_group_check=True,
                        )

        def kproj(mt):
            msl = ts(mt, 128)
            kps = pa_ps.tile([128, 1024], F32, tag="proj")
            # K projection in fp8 DoubleRow: contraction pairs of 128-chunks
            # (effective K=256 per matmul, ~1.4-2x PE throughput); fp8 on
            # Q/K is accuracy-safe because the normalizer z cancels most of
            # the quantization error (measured rel 0.0069 vs 0.02 budget).
            for j in range(4):
                for o in range(2):
                    nc.tensor.matmul(
                        kps[:, ts(o, 512)],
                        X8[:, 2 * j:2 * j + 2, msl],
                        wsb["wk8"][:, 2 * j:2 * j + 2, ts(o, 512)],
                        start=(j == 0), stop=(j == 3),
                        perf_mode=DR,
                    )
            r1 = pa_tmp.tile([128, 1024], DT, tag="r1", bufs=4)
            nc.scalar.activation(r1, kps, AF.Relu)
            m1 = pa_tmp.tile([128, 1024], DT, tag="m1", bufs=4)
            nc.vector.tensor_scalar_min(m1, kps, 0.0)
            e1 = pa_tmp.tile([128, 1024], DT, tag="e1", bufs=4)
            nc.scalar.activation(e1, m1, AF.Exp)
            ksb = pa_tmp.tile([128, 1024], DT, tag="ksb", bufs=KAH + 2)
            nc.gpsimd.tensor_add(ksb, r1, e1)
            ksb_hist[mt] = ksb

        # K run-ahead: X8a+wk8 (2MB) land ~13us before Xh1+wv (4MB more),
        # so run the first KAH m-tiles' K projections (all inside X8a's
        # token half) while V inputs are still in flight -- the PE never
        # sees the DMA gap and HAM stays warm.
        KAH = 8
        for mt in range(KAH):
            kproj(mt)
            if mt == 1:
                # phase-B inputs: issued after the first m-tile's matmuls so
                # phase-A waits never count these transfers.
                bg_ap = bg_d.ap()
                nc.sync.dma_start(
                    out=bg_sb[:],
                    in_=bass.AP(tensor=bg_ap.tensor, offset=0,
                                ap=[[1, 128], [128, 8]]),
                )
                nc.sync.dma_start(out=sel[:], in_=sel_d.ap())
                for nm in ("wq8", "wg8", "wo"):
                    big_dma(wsb[nm][:, :, :], w_in[nm], DIM, 0, DIM)

        for mt in range(NMT):
            msl = ts(mt, 128)
            vps = pa_ps.tile([128, 16, 64], F32, tag="proj")
            for i in range(8):
                for o in range(2):
                    nc.tensor.matmul(
                        vps[:, ts(o, 8), :],
                        mm(X[:, i, msl]),
                        mm(wsb["wv"][:, i, ts(o, 512)]),
                        start=(i == 0), stop=(i == 7),
                    )
            vp = pa_tmp.tile([128, 16, 65], DT, tag="vp")
            nc.vector.memset(vp[:, :, 64:65], 1.0)
            nc.scalar.copy(vp[:, :, 0:64], vps[:, :, :])
            vp_hist[mt] = vp

            if mt > 0:
                kv_mms(mt - 1)
            # kproj after kv: pushes kproj(KAH)'s X8b dependency ~1us past
            # the X8b DMA completion so the first V iteration never stalls
            if mt + KAH < NMT:
                kproj(mt + KAH)
        kv_mms(NMT - 1)

        kv_sb = pa_tmp.tile([128, 8, 65], F32, tag="kv_sb", bufs=1,
                            name="kv_sb")
        for w in range(2):
            nc.vector.tensor_copy(kv_sb[:, 4 * w:4 * w + 4, :],
                                  kv_ps[w][:, :, 0:65])
        nc.sync.dma_start(out=cc_in.ap()[:, :, :], in_=kv_sb[:])

    nc.gpsimd.collective_compute(
        "AllReduce",
        mybir.AluOpType.add,
        replica_groups=[[0, 1], [2, 3], [4, 5], [6, 7]],
        ins=[cc_in.ap().opt()],
        outs=[cc_out.ap().opt()],
    )

    # ---------------- phase B ----------------
    with (
        tc.tile_pool(name="pb_tmp", bufs=2) as pb_tmp,
        tc.tile_pool(name="pb_small", bufs=1) as pb_small,
        tc.tile_pool(name="pb_qg", bufs=1) as pb_qg,
    ):
        # collective results live in the persist pool: fresh SBUF, so the
        # kvf DMA has no write-after-read wait on phase-A consumers.
        kvf = persist.tile([128, 8, 65], F32, tag="kvf")
        kvb = persist.tile([128, 8, 65], DT, tag="kvb")
        ksd = persist.tile([128, 8, 16], DT, tag="ksd")

        ps_proj_cm = tc.tile_pool(name="ps_proj", bufs=6, space="PSUM")
        ps_proj = ps_proj_cm.__enter__()

        def proj_block(p, csl, qsb, gsb, which):
            pps = ps_proj.tile([128, CHUNK], F32, tag="proj")
            wname = "wq8" if which == "q" else "wg8"
            # fp8 DoubleRow (see K projection note; G measured rel 0.0127
            # alone, 0.0162 combined with Q/K — inside the 0.02 budget)
            for j in range(4):
                nc.tensor.matmul(
                    pps, wsb[wname][:, 2 * j:2 * j + 2, ts(p, 128)],
                    X8[:, 2 * j:2 * j + 2, csl],
                    start=(j == 0), stop=(j == 3),
                    perf_mode=DR,
                )
            if which == "q":
                # relu on DVE, not ACT: with fp8 projections the PE pace is
                # ~0.85us/block and an ACT-side relu+exp chain (1.2us) makes
                # pass 1 ACT-bound (PSUM-reuse stalls + HAM oscillation).
                # bf16 intermediates + bufs=4: the serial MIN->EXP->ADD
                # chain (~2.7us fp32) is longer than the 1.73us/pair PE
                # cadence, so with bufs=2 the r1/e1 recycling stalled the
                # PE ~0.8us every third pair and left a ~9us drain tail.
                r1 = pb_tmp.tile([128, CHUNK], DT, tag="br1", bufs=4)
                nc.vector.tensor_scalar_max(r1, pps, 0.0)
                m1 = pb_tmp.tile([128, CHUNK], DT, tag="bm1", bufs=4)
                nc.vector.tensor_scalar_min(m1, pps, 0.0)
                e1 = pb_tmp.tile([128, CHUNK], DT, tag="be1", bufs=4)
                nc.scalar.activation(e1, m1, AF.Exp)
                nc.gpsimd.tensor_add(qsb[:, p, :], r1, e1)
            else:
                # sigmoid(t) = 0.5*(1+tanh(t/2)): Tanh lives in the SAME
                # ACT table set as Exp (exp_and_others), so pass 1 runs with
                # ZERO table swaps (Sigmoid's set forced a 1.28us
                # ACT_TABLE_LOAD per Q/G pair). Host pre-halves bg; the
                # 0.5*(1+...) is folded into zqr and a fused DVE op in
                # pass 2.
                nc.scalar.activation(gsb[:, p, :], pps, AF.Tanh,
                                     bias=bg_sb[:, p:p + 1], scale=0.5)

        # ---- pass 1: Q and G projections for ALL chunks (collective-free
        # PE work that covers the AllReduce round-trip) ----
        qsbs, gsbs = [], []
        for ch in range(NCH):
            csl = ts(ch, CHUNK)
            qsb = pb_qg.tile([128, 8, CHUNK], DT, tag=f"qsb{ch}")
            gsb = pb_qg.tile([128, 8, CHUNK], DT, tag=f"gsb{ch}")
            qsbs.append(qsb)
            gsbs.append(gsb)
            # Q and G blocks interleaved: a straight 8-block Q run outpaces
            # DVE (2 relu/min ops per Q block, none per G block) and stalls
            # the PE on PSUM-reuse every chunk; per Q+G pair the PE takes
            # 1.7us vs DVE's 1.38us, so the pair loop never stalls.
            # LAST chunk runs all Q blocks first: the Q elu chains drain
            # under the G projections, so pass 1 ends on cheap tanh ops
            # instead of a multi-us chain tail that stalls pass 2's start.
            if ch == NCH - 1:
                for p in range(8):
                    proj_block(p, csl, qsb, gsb, "q")
                for p in range(8):
                    proj_block(p, csl, qsb, gsb, "g")
            else:
                for p in range(8):
                    proj_block(p, csl, qsb, gsb, "q")
                    proj_block(p, csl, qsb, gsb, "g")
                    if ch == 0 and p == 0:
                        nc.sync.dma_start(out=kvf[:],
                                          in_=cc_out.ap()[:, :, :])

        ps_proj_cm.__exit__(None, None, None)

        # ---- collective prep + scheduling gate, ALL on the Pool engine.
        # No pass-1 matmul ever waits on Pool completions, so these
        # collective-dependent ops cannot poison pass-1 counting-semaphore
        # thresholds (they did when placed on ACT or DVE). ksd2/kvb2 gain a
        # zero-valued dependency on the LAST projection block's output:
        # the Tile scheduler's readiness model ignores AllReduce latency
        # and would otherwise hoist the first collective-dependent matmul
        # right behind chunk 0's projections, head-blocking the in-order
        # PE queue on the collective.
        # Every prep op that reads kvf is ALSO data-gated (via a zero add
        # operand) on chunk 2's projections: an ungated op here gets
        # emitted mid-Pool-queue by the scheduler's optimistic model and
        # head-blocks Pool on the AllReduce, which stalls pass-1 DVE relu
        # ops (they WAR-wait Pool's qsb-add drains) and thus the PE.
        # Gating on chunk 2 (75% of pass 1) keeps ~80us of collective
        # coverage while leaving the final chunk to hide this prep.
        # ksd carries plain k_sum (SCALE folded into the z chain) so the
        # scatter can be 2-operand gated adds.
        prep_prio = tc.high_priority(offset=-1000000)
        prep_prio.__enter__()
        zlg = pb_small.tile([128, 65], F32, tag="zlg")
        nc.gpsimd.tensor_scalar_mul(zlg, gsbs[2][:, 7, 0:65], 0.0)
        nc.gpsimd.memset(ksd[:], 0.0)

        def _b8(apx, inner):
            # broadcast a [128, inner] AP across the 8 head-pair slices
            return bass.AP(tensor=apx.tensor, offset=apx.offset,
                           ap=[apx.ap[0], [0, 8], [1, inner]])

        # kvb = kvf + 0 (fp32 -> bf16 cast) in ONE Pool op
        nc.gpsimd.tensor_add(kvb[:, :, :], kvf[:, :, :], _b8(zlg[:, :], 65))
        # k_sum scatter: column 64 of head-pair p lands block-diagonally at
        # ksd[, p, 2p] (partitions 0:64) / [, p, 2p+1] (64:128) -- one
        # strided op per half instead of 16 scalar copies
        for half, coff in ((slice(0, 64), 0), (slice(64, 128), 1)):
            dsl = ksd[half, :, :]
            dst = bass.AP(tensor=dsl.tensor, offset=dsl.offset + coff,
                          ap=[dsl.ap[0], [18, 8], [1, 1]])
            ssl = kvf[half, :, 64:65]
            src = bass.AP(tensor=ssl.tensor, offset=ssl.offset,
                          ap=[ssl.ap[0], [65, 8], [1, 1]])
            zsl = zlg[half, 0:1]
            zb = bass.AP(tensor=zsl.tensor, offset=zsl.offset,
                         ap=[zsl.ap[0], [0, 8], [1, 1]])
            nc.gpsimd.tensor_add(dst, src, zb)
        ksd2 = persist.tile([128, 8, 16], DT, tag="ksd2")
        kvb2 = persist.tile([128, 8, 65], DT, tag="kvb2")
        zl65 = pb_small.tile([128, 65], DT, tag="zl65")
        # gate on the last chunk's SECOND Q block (early in its Q-first
        # run): prep stage 2 + chunk 0's qk/z chain then complete under the
        # last chunk's remaining ~12 projection blocks, so pass 2's first
        # ops matmul can issue the moment pass 1 drains. Collective
        # coverage is still ~45us of projection work.
        nc.gpsimd.tensor_scalar_mul(zl65, qsbs[NCH - 1][:, 1, 0:65], 0.0)
        nc.gpsimd.tensor_add(ksd2[:, :, :], ksd[:, :, :],
                             _b8(zl65[:, 0:16], 16))
        nc.gpsimd.tensor_add(kvb2[:, :, :], kvb[:, :, :],
                             _b8(zl65[:, :], 65))
        prep_prio.__exit__(None, None, None)

        ps_ops_cm = tc.tile_pool(name="ps_ops", bufs=2, space="PSUM")
        ps_z_cm = tc.tile_pool(name="ps_z", bufs=2, space="PSUM")
        ps_qk_cm = tc.tile_pool(name="ps_qk", bufs=2, space="PSUM")
        ps_y_cm = tc.tile_pool(name="ps_y", bufs=2, space="PSUM")
        ps_ops = ps_ops_cm.__enter__()
        ps_z = ps_z_cm.__enter__()
        ps_qk = ps_qk_cm.__enter__()
        ps_y = ps_y_cm.__enter__()

        # ---- pass 2: attention + output projection per chunk; each
        # chunk's qk runs one chunk ahead so the z reciprocal chain (DVE)
        # hides under the previous chunk's y matmuls. The whole pass is
        # deprioritized far past pass 1 so the scheduler's ready-heap
        # always prefers projection matmuls over collective-dependent ones.
        prio_cm = tc.high_priority(offset=-1000000)
        prio_cm.__enter__()

        def qk_mms(ch):
            qkps = ps_qk.tile([16, CHUNK], F32, tag="qk")
            for p in range(8):
                nc.tensor.matmul(
                    qkps, mm(ksd2[:, p, :]), mm(qsbs[ch][:, p, :]),
                    start=(p == 0), stop=(p == 7),
                    skip_group_check=True,
                )
            return qkps

        def y_dblock(d, asb_o, csl_o):
            yps = ps_y.tile([128, CHUNK], F32, tag="y")
            for fi in range(8):
                nc.tensor.matmul(
                    yps, mm(wsb["wo"][:, fi, ts(d, 128)]),
                    mm(asb_o[:, fi, :]),
                    start=(fi == 0), stop=(fi == 7),
                )
            ysb = pb_tmp.tile([128, CHUNK], DT, tag="ysb")
            nc.scalar.copy(ysb, yps)
            nc.sync.dma_start(out=y_d.ap()[ts(d, 128), csl_o],
                              in_=ysb[:])

        # software-pipelined: chunk ch's ops/gate blocks interleave with
        # chunk ch-1's output-projection d-blocks, so the asb DVE chain
        # (~1.3us/block, longer than the 0.65us/block ops-phase PE pace)
        # hides entirely under the previous chunk's y matmuls instead of
        # stalling the PE on ps_ops recycling.
        qkps_next = qk_mms(0)
        prev = None
        for ch in range(NCH):
            csl = ts(ch, CHUNK)
            qsb, gsb = qsbs[ch], gsbs[ch]
            qkps = qkps_next
            zq = pb_tmp.tile([16, CHUNK], F32, tag="zq")
            nc.vector.tensor_scalar_max(zq, qkps, CLAMP)
            zr = pb_tmp.tile([16, CHUNK], F32, tag="zr")
            # full-precision DVE reciprocal measured 3.3us for [16,512] and
            # sat on chunk 0's critical path; approx_fast is ~51 ULP
            # (rel ~4e-6, way inside the 2e-2 budget) at ~5x the speed.
            # zq >= CLAMP=1e-6 so the undefined edge cases can't occur.
            nc.vector.reciprocal_approx_fast(zr, zq)
            zqr = pb_tmp.tile([16, CHUNK], DT, tag="zqr")
            # extra 0.5: the gate is (1+tanh)/2; the /2 lives here
            nc.vector.tensor_scalar_mul(zqr, zr, SCALE * 0.5)

            asb = pb_big.tile([128, 8, CHUNK], DT, tag="asb")
            for p in range(8):
                ops_ = ps_ops.tile([128, CHUNK], F32, tag="ops")
                for rr in range(2):
                    pr = slice(64 * rr, 64 * rr + 64)
                    nc.tensor.matmul(
                        ops_[pr, :], mm(kvb2[pr, p, 0:64]),
                        mm(qsb[pr, p, :]),
                        start=True, stop=True,
                    )
                zbps = ps_z.tile([128, CHUNK], F32, tag="z")
                nc.tensor.matmul(zbps, mm(sel[:, p, :]), mm(zqr),
                                 start=True, stop=True)
                t1 = pb_tmp.tile([128, CHUNK], DT, tag="bt1")
                # each op reads at most one PSUM operand (HW restriction);
                # fused (gsb + 1) * ops in a single DVE instruction
                nc.vector.scalar_tensor_tensor(
                    t1, gsb[:, p, :], 1.0, ops_,
                    op0=mybir.AluOpType.add, op1=mybir.AluOpType.mult)
                nc.vector.tensor_mul(asb[:, p, :], t1, zbps)
                if p == 1 and ch + 1 < NCH:
                    qkps_next = qk_mms(ch + 1)
                if prev is not None:
                    y_dblock(p, prev[0], prev[1])
            prev = (asb, csl)

        for d in range(8):
            y_dblock(d, prev[0], prev[1])

        prio_cm.__exit__(None, None, None)
        ps_y_cm.__exit__(None, None, None)
        ps_qk_cm.__exit__(None, None, None)
        ps_z_cm.__exit__(None, None, None)
        ps_ops_cm.__exit__(None, None, None)


def _np_dt(dt_mode):
    return ml_dtypes.bfloat16 if dt_mode == "bf16" else np.float32


def prep_inputs(x, Wq, Wk, Wv, Wg, bg, Wo, dt_mode=DT_MODE):
    npdt = _np_dt(dt_mode)
    f8 = ml_dtypes.float8_e4m3
    x_f = np.ascontiguousarray(np.asarray(x, np.float32).reshape(B * N, DIM))
    w_t = {}
    w_t["wv"] = np.ascontiguousarray(
        np.asarray(Wv, np.float32).T).astype(npdt)
    for nm, W in (("wq8", Wq), ("wk8", Wk), ("wg8", Wg)):
        w_t[nm] = np.ascontiguousarray(
            np.asarray(W, np.float32).T).astype(f8)
    w_t["wo"] = np.ascontiguousarray(
        np.asarray(Wo, np.float32).T).astype(npdt)
    # halved: the gate is computed as tanh(0.5*g + 0.5*bg) (sigmoid via
    # tanh keeps ACT on the exp_and_others table set -- no table swaps)
    bg_f = np.ascontiguousarray(np.asarray(bg, np.float32) * 0.5)
    in_maps = []
    for c in range(N_CORES):
        xt_T = np.ascontiguousarray(x_f[c * TPC:(c + 1) * TPC].T)
        m = {"xt": xt_T.astype(npdt), "x8": xt_T.astype(f8), "bg": bg_f}
        m.update(w_t)
        in_maps.append(m)
    return in_maps


def unshard_output(y_parts):
    out = np.empty((B * N, DIM), np.float32)
    for c in range(N_CORES):
        out[c * TPC:(c + 1) * TPC] = np.asarray(y_parts[c], np.float32).T
    return out.reshape(B, N, DIM)


def get_nc(dt_mode=DT_MODE):
    key = ("nc", dt_mode)
    if key not in _CACHE:
        _CACHE[key] = _build(dt_mode)
    return _CACHE[key]


def kernel(x, Wq, Wk, Wv, Wg, bg, Wo):
    from concourse.bass_utils import run_bass_kernel_spmd

    nc = get_nc()
    in_maps = prep_inputs(x, Wq, Wk, Wv, Wg, bg, Wo)
    res = run_bass_kernel_spmd(nc, in_maps, core_ids=list(range(N_CORES)))
    return unshard_output([res.results[c]["y"] for c in range(N_CORES)])

